# revision 1
# baseline (speedup 1.0000x reference)
"""Multi-head attention + out-proj + residual + LayerNorm on 8 trn2 cores.

Sharding: (batch, seq-half) -> 8 shards, collective-free. Each core gets
transposed activations (host-prepped) plus shared (transposed) weights and
computes its full [1024, 1024] output block.

Design (764us baseline -> ~413us): everything stays in SBUF (no DRAM
staging); the softmax normalize chain runs entirely off the TensorE queue
(full-tile PSUM copy FIRST to free the PV accumulator immediately, then
recip_approx_fast on DVE + partition_broadcast on gpsimd + one DVE mul)
so the PE FIFO never head-of-line blocks and HAM stays warm; K/Q/V
projection matmul groups are pumped into the attention sk-loop so the PE
has independent work while ACT chews the exp stream (the head's V/K0/Q0
work is deadline-scheduled into attn(0,0) with consumption-ordered DMA
chunks, putting the first exp ~23us in); PV runs in fp8e4 DoubleRow (256
keys contracted per matmul); LN rstd is a DVE-only Newton rsqrt (avoids
ACT Exp<->Sqrt table thrash); the LN output stage is bf16 end-to-end
(host upcasts) and the full final chains for the first query half are
pumped into the last attention call.

  phase V: V_all[Sk, H, 1+dv] fp8 in SBUF (ones col first = softmax denom)
  loop c (head pair): K/Q proj for c+1 + V heads 8-15 pumped into attn(c)
  attn  : scoresT[Sk,Sq] bf16 row-tiled head pair -> exp (ACT, fp8 pairs)
          OT[1+dv, Sq] += [1|V_h].T @ expT  (DoubleRow, row 0 = denom)
          epilogue: recip_fast -> gpsimd bcast -> DVE mul -> SBUF dma
  final : out = LN(concatT.T @ WpT + q_res) * scale + offset
"""

import os
from contextlib import ExitStack

import numpy as np

import concourse.bass as bass
import concourse.tile as tile
from concourse import bacc, mybir
from concourse._compat import with_exitstack
from concourse.bass_utils import run_bass_kernel_spmd

B, S, D = 4, 2048, 1024
H, DK, DV = 16, 64, 64
F = H * DV            # 1024 flattened head dim (== H*DK)
N_CORES = 8
SQ = S // 2           # 1024 queries per core
SK = S                # 2048 keys per core
P = 128
KD = D // P           # 8 contraction chunks over d_model
NF = F // P           # 8 head-pair chunks
NSK = SK // P         # 16 key chunks
TEMP = float(np.sqrt(D))
EPS = 1e-9

F32 = mybir.dt.float32
BF16 = mybir.dt.bfloat16
F8 = mybir.dt.float8e4

# sk indices at which one pumped proj group is emitted
PUMP_SKS = (0, 3, 6, 9, 12)
PUMP_SKS_LAST = (0, 4, 8, 12)

LAST_RESULT = None    # BassKernelResults of the most recent kernel() call


@with_exitstack
def _mha_kernel(ctx: ExitStack, tc: tile.TileContext, out_ap, ins):
    nc = tc.nc
    AF = mybir.ActivationFunctionType
    ALU = mybir.AluOpType

    xq_r = ins["qT"].rearrange("(c p) s -> p c s", p=P)
    xk_r = ins["kT"].rearrange("(c p) s -> p c s", p=P)
    xv_r = ins["vT"].rearrange("(c p) s -> p c s", p=P)

    resident = ctx.enter_context(tc.tile_pool(name="resident", bufs=1))
    # V_all with a ones column appended per head: [sk_part, sk, head, 65]
    v_sb = resident.tile([P, NSK, H, 65], F8)
    nc.vector.memset(v_sb[:, :, :, 0:1], 1.0)
    # concat.T output of attention: partition = f%128, [128, chunk, q]
    ot_sb = resident.tile([P, NF, SQ], BF16)

    with (
        tc.tile_pool(name="scps", bufs=2, space="PSUM") as scps,
        tc.tile_pool(name="otps", bufs=2, space="PSUM") as otps,
        tc.tile_pool(name="ktp", bufs=2) as ktp,
        tc.tile_pool(name="qtp", bufs=2) as qtp,
        tc.tile_pool(name="expp", bufs=2) as expp,
        tc.tile_pool(name="rcp", bufs=2) as rcp,
        tc.tile_pool(name="bcp", bufs=2) as bcp,
        tc.tile_pool(name="oop", bufs=2) as oop,
    ):
        kt_tiles = {}
        qt_tiles = {}

        def attn(c, sq, work, pump_sks=PUMP_SKS, sched=None):
            """Attention for head-pair chunk c, query half sq (512 q).

            work: list of emit-closures (projection slivers). With
            pump_sks=None they are spread evenly across the sk steps so
            no insertion starves the score->exp chain; a tuple pins
            pumping to those sk indices (used for the heavier final_mm
            groups in the last block).
            """
            ktc = kt_tiles[c]
            qtc = qt_tiles[c]
            ot_ps = [otps.tile([65, 512], F32, tag="ot", name="otp")
                     for _ in range(2)]

            def emit_scores(sk, reps=1):
                # reps>1 emits numerically-invisible duplicate matmuls
                # (each start=True overwrite) as HAM warm-keeping filler
                # when the pump has no real work for this slot
                sc = scps.tile([P, 2, 512], F32, tag="sc", name="sc")
                for _ in range(reps):
                    for hh in range(2):
                        base = hh * 64
                        nc.tensor.matmul(
                            sc[:, hh, :],
                            lhsT=ktc[base:base + 64, sk * P:(sk + 1) * P],
                            rhs=qtc[base:base + 64, sq * 512:(sq + 1) * 512],
                            start=True,
                            stop=True,
                        )
                return sc

            sc_prev = emit_scores(0)
            ex = None
            reps = 1
            for sk in range(NSK):
                par = sk % 2
                if par == 0:
                    # fp8 exp pairs: chunk parity on its own axis so PV can
                    # contract 256 keys per DoubleRow matmul
                    ex = expp.tile([P, 2, 2, 512], F8, tag="ex", name="ex")
                nc.scalar.activation(ex[:, :, par, :], sc_prev, AF.Exp,
                                     scale=1.0 / TEMP)
                if sk + 1 < NSK:
                    sc_prev = emit_scores(sk + 1, reps=reps)
                reps = 1
                if sched is not None:
                    for w in sched.get(sk, []):
                        w()
                elif sk in pump_sks:
                    if work:
                        work.pop(0)()
                    else:
                        reps = 3
                if par == 1:
                    dj = sk // 2
                    for hh in range(2):
                        nc.tensor.matmul(
                            ot_ps[hh],
                            lhsT=v_sb[:, 2 * dj:2 * dj + 2, 2 * c + hh, :],
                            rhs=ex[:, hh, :, :],  # [128, 2, 512]
                            start=(dj == 0),
                            stop=(dj == NSK // 2 - 1),
                            perf_mode=mybir.MatmulPerfMode.DoubleRow,
                        )
            # epilogue: normalize rows 1:65 by denom row 0, no PE
            # involvement (custom DVE ops require partition-0 APs).
            # The full-tile copy costs the same DVE time as a 1-row copy
            # (cost ~ free-size/partition) but frees the PSUM accumulator
            # immediately, so the next iteration's first PV matmul never
            # head-of-line blocks the PE queue on this chain.
            for hh in range(2):
                otb = rcp.tile([65, 512], F32, tag="otb")
                nc.vector.tensor_copy(otb, ot_ps[hh])
                rc = rcp.tile([1, 512], F32, tag="rc")
                nc.vector.reciprocal_approx_fast(rc, otb[0:1, :])
                bc = bcp.tile([65, 512], F32, tag="bc")
                nc.gpsimd.partition_broadcast(bc, rc)
                oo = oop.tile([65, 512], BF16, tag="oo")
                nc.vector.tensor_mul(oo, otb, bc)
                nc.sync.dma_start(
                    ot_sb[hh * 64:(hh + 1) * 64, c, sq * 512:(sq + 1) * 512],
                    oo[1:65, :],
                )

        with tc.tile_pool(name="wkq", bufs=1) as wkq:
            wk = wkq.tile([P, KD, F], BF16)
            xk = wkq.tile([P, KD, SK], BF16)
            wq = wkq.tile([P, KD, F], BF16)
            xq = wkq.tile([P, KD, SQ], BF16)

            wv = wkq.tile([P, KD, F], BF16)
            xv = wkq.tile([P, KD, SK], BF16)
            # DMAs ordered by first consumption: the critical prefix
            # (K0/Q0 first column groups + first V key pair) lands in
            # ~2.5MB so the first scores/exp issue ~8us in; the rest of
            # the head work is deadline-scheduled into attn(0,0)
            wv_r = ins["wvT"].rearrange("(c p) f -> p c f", p=P)
            wk_r = ins["wkT"].rearrange("(c p) f -> p c f", p=P)
            wq_r = ins["wqT"].rearrange("(c p) f -> p c f", p=P)
            nc.sync.dma_start(wk[:, :, 0:P], wk_r[:, :, 0:P])
            nc.sync.dma_start(xk[:, :, 0:512], xk_r[:, :, 0:512])
            nc.sync.dma_start(wq[:, :, 0:P], wq_r[:, :, 0:P])
            nc.sync.dma_start(xq[:, :, 0:512], xq_r[:, :, 0:512])
            nc.sync.dma_start(wv[:, :, 0:512], wv_r[:, :, 0:512])
            for j in range(2):
                nc.sync.dma_start(xv[:, :, j * 256:(j + 1) * 256],
                                  xv_r[:, :, j * 256:(j + 1) * 256])
            nc.sync.dma_start(xk[:, :, 512:1024], xk_r[:, :, 512:1024])
            nc.sync.dma_start(xv[:, :, 512:768], xv_r[:, :, 512:768])
            nc.sync.dma_start(xk[:, :, 1024:1536], xk_r[:, :, 1024:1536])
            nc.sync.dma_start(xv[:, :, 768:1024], xv_r[:, :, 768:1024])
            nc.sync.dma_start(xk[:, :, 1536:2048], xk_r[:, :, 1536:2048])
            for j in range(4, 8):
                nc.sync.dma_start(xv[:, :, j * 256:(j + 1) * 256],
                                  xv_r[:, :, j * 256:(j + 1) * 256])
            nc.sync.dma_start(xq[:, :, 512:1024], xq_r[:, :, 512:1024])
            nc.sync.dma_start(wv[:, :, 512:1024], wv_r[:, :, 512:1024])
            nc.sync.dma_start(wk[:, :, P:F], wk_r[:, :, P:F])
            nc.sync.dma_start(wq[:, :, P:F], wq_r[:, :, P:F])

            with tc.tile_pool(name="projps2", bufs=2, space="PSUM") as pps:

                # HAM warmup: ~10 dummy matmuls at t~0 (no DMA dependency)
                # push the PE past the 3.4us activity window so the first
                # real projections run at 2.4GHz instead of the cold 1.2
                warm_sb = wkq.tile([P, 512], BF16)
                nc.vector.memset(warm_sb, 0.0)
                warm_ps = pps.tile([P, 512], F32, tag="ps", name="warm")
                for _ in range(10):
                    nc.tensor.matmul(
                        warm_ps[0:64, :],
                        lhsT=warm_sb[0:64, 0:64],
                        rhs=warm_sb[0:64, :],
                        start=True,
                        stop=True,
                    )

                def _slivers(mk_mm, copy_out, nuke=KD):
                    # split one 8-matmul psum group into 2-matmul slivers
                    st = {}

                    def sliver(i):
                        def emit():
                            if i == 0:
                                st["ps"] = pps.tile([P, 512], F32,
                                                    tag="ps", name="ps")
                            for kd in range(nuke * i, nuke * (i + 1)):
                                mk_mm(st["ps"], kd)
                            if i == KD // nuke - 1:
                                copy_out(st["ps"])
                        return emit
                    return [sliver(i) for i in range(KD // nuke)]

                def v_group(n, sk):
                    def mk_mm(ps, kd):
                        nc.tensor.matmul(
                            ps,
                            lhsT=xv[:, kd, sk * P:(sk + 1) * P],
                            rhs=wv[:, kd, n * 512:(n + 1) * 512],
                            start=(kd == 0),
                            stop=(kd == KD - 1),
                        )

                    def copy_out(ps):
                        nc.vector.tensor_copy(
                            v_sb[:, sk, n * 8:(n + 1) * 8, 1:65],
                            ps.rearrange("p (h e) -> p h e", h=8),
                        )
                    return _slivers(mk_mm, copy_out)

                # (V heads 0-7 are deadline-scheduled into attn(0,0))

                def k_group(c, n):
                    def mk_mm(ps, kd):
                        if c not in kt_tiles:
                            kt_tiles[c] = ktp.tile([P, SK], BF16, tag="kt",
                                                   name="ktc")
                        nc.tensor.matmul(
                            ps,
                            lhsT=wk[:, kd, c * P:(c + 1) * P],
                            rhs=xk[:, kd, n * 512:(n + 1) * 512],
                            start=(kd == 0),
                            stop=(kd == KD - 1),
                        )

                    def copy_out(ps):
                        nc.vector.tensor_copy(
                            kt_tiles[c][:, n * 512:(n + 1) * 512], ps)
                    return _slivers(mk_mm, copy_out)

                def q_group(c, n):
                    def mk_mm(ps, kd):
                        if c not in qt_tiles:
                            qt_tiles[c] = qtp.tile([P, SQ], BF16, tag="qt",
                                                   name="qtc")
                        nc.tensor.matmul(
                            ps,
                            lhsT=wq[:, kd, c * P:(c + 1) * P],
                            rhs=xq[:, kd, n * 512:(n + 1) * 512],
                            start=(kd == 0),
                            stop=(kd == KD - 1),
                        )

                    def copy_out(ps):
                        nc.vector.tensor_copy(
                            qt_tiles[c][:, n * 512:(n + 1) * 512], ps)
                    return _slivers(mk_mm, copy_out)

                if True:
                    # minimal upfront head: only what attn(0,0)'s first
                    # emitted instructions read
                    k_group(0, 0)[0]()
                    q_group(0, 0)[0]()
                    for s in v_group(0, 0):
                        s()
                    for s in v_group(0, 1):
                        s()
                    # remaining head work, scheduled so each group is
                    # emitted strictly before its first reader's emission
                    # (scores(sk+1) emit at step sk; PV(dj) at step 2dj+1)
                    head_sched = {
                        0: [v_group(0, 2)[0], v_group(0, 3)[0]],
                        1: [k_group(0, 1)[0]],
                        2: [v_group(0, 4)[0], v_group(0, 5)[0]],
                        3: [v_group(0, 6)[0]],
                        4: [v_group(0, 7)[0], k_group(0, 2)[0]],
                        5: [v_group(0, 8)[0]],
                        6: [v_group(0, 9)[0]],
                        7: [v_group(0, 10)[0]],
                        8: [v_group(0, 11)[0], k_group(0, 3)[0]],
                        9: [v_group(0, 12)[0]],
                        10: [v_group(0, 13)[0]],
                        11: [v_group(0, 14)[0]],
                        12: [v_group(0, 15)[0]],
                        13: [q_group(0, 1)[0]],
                    }
                    # main loop: attention on c, proj groups for c+1 and
                    # V n=1 slices pumped into the sk streams
                    for c in range(NF - 1):
                        work = []
                        for n in range(SK // 512):
                            work += k_group(c + 1, n)
                        for n in range(SQ // 512):
                            work += q_group(c + 1, n)
                        if c < 4:
                            for j in range(4):
                                work += v_group(1, 4 * c + j)
                        if c == 0:
                            attn(0, 0, [], sched=head_sched)
                            attn(0, 1, work,
                                 pump_sks=(0, 2, 4, 6, 8, 10, 12, 14))
                        else:
                            attn(c, 0, work)
                            attn(c, 1, work)
                        for w in work:   # leftovers: consumers are in
                            w()          # later blocks, so safe

        # ---------------- final: out proj + residual + layernorm -----------
        with (
            tc.tile_pool(name="wpp", bufs=1) as wpp,
            tc.tile_pool(name="lnc", bufs=1) as lnc,
            tc.tile_pool(name="qres", bufs=8) as qrp,
            tc.tile_pool(name="lnx", bufs=9) as lnx,
            tc.tile_pool(name="lnxn", bufs=3) as lnxn,
            tc.tile_pool(name="stat", bufs=32) as stp,
            tc.tile_pool(name="fps", bufs=2, space="PSUM") as fps,
        ):
            wp = wpp.tile([P, NF, D], BF16)
            nc.sync.dma_start(wp, ins["wpT"].rearrange("(c p) f -> p c f", p=P))
            scale_sb = lnc.tile([P, 2, 512], BF16)
            nc.sync.dma_start(
                scale_sb, ins["scale_b"].rearrange("p (a b) -> p a b", a=2))
            offset_sb = lnc.tile([P, 2, 512], BF16)
            nc.sync.dma_start(
                offset_sb, ins["offset_b"].rearrange("p (a b) -> p a b", a=2))

            parts = {}

            def final_mm(qc, warm=0):
                def emit():
                    qr = qrp.tile([P, 2, 512], BF16, tag="qr")
                    nc.sync.dma_start(
                        qr,
                        ins["qres"][qc * P:(qc + 1) * P, :].rearrange(
                            "p (a b) -> p a b", a=2),
                    )
                    x = lnx.tile([P, 2, 512], F32, tag="x", name="x")
                    for d in range(2):
                        fp = fps.tile([P, 512], F32, tag="fp")
                        # warm>0: duplicate the f=0 matmul as HAM filler;
                        # the real accumulation below start=True-overwrites,
                        # so these are numerically invisible. They have no
                        # dependency on ot chunk 7, so they execute inside
                        # the last epilogue's wait window and keep the PE
                        # clock at 2.4GHz for the real tail matmuls.
                        for _ in range(warm):
                            nc.tensor.matmul(
                                fp,
                                lhsT=ot_sb[:, 0, qc * P:(qc + 1) * P],
                                rhs=wp[:, 0, d * 512:(d + 1) * 512],
                                start=True,
                                stop=True,
                            )
                        for f in range(NF):
                            nc.tensor.matmul(
                                fp,
                                lhsT=ot_sb[:, f, qc * P:(qc + 1) * P],
                                rhs=wp[:, f, d * 512:(d + 1) * 512],
                                start=(f == 0),
                                stop=(f == NF - 1),
                            )
                        nc.vector.tensor_add(x[:, d, :], fp, qr[:, d, :])
                    stats = stp.tile([P, 2, 6], F32, tag="st", name="st")
                    for gsub in range(2):
                        nc.vector.bn_stats(stats[:, gsub, :], x[:, gsub, :])
                    mv = stp.tile([P, 2], F32, tag="mv", name="mv")
                    nc.vector.bn_aggr(mv, stats)
                    parts[qc] = (x, mv)
                return emit

            def final_ln(qc):
                # rstd = rsqrt(var*D/(D-1)) via DVE-only Newton (seed 1/v):
                # avoids the ACT Sqrt table-set thrash against Exp. EPS=1e-9
                # vanishes in fp32 rounding for std ~ O(1) (reference rounds
                # identically). var~1 here so 3 iterations reach ~1e-6 rel.
                x, mv = parts[qc]
                v = stp.tile([P, 1], F32, tag="v", name="v")
                nc.vector.tensor_scalar_mul(v, mv[:, 1:2],
                                            float(D) / float(D - 1))
                y = stp.tile([P, 1], F32, tag="y", name="y")
                nc.vector.reciprocal(y, v)
                for _ in range(2):
                    t = stp.tile([P, 1], F32, tag="t", name="t")
                    nc.vector.tensor_mul(t, y, y)
                    nc.vector.tensor_mul(t, t, v)
                    nc.vector.tensor_scalar(t, t, -0.5, 1.5,
                                            ALU.mult, ALU.add)
                    nc.vector.tensor_mul(y, y, t)
                xn = lnxn.tile([P, 2, 512], BF16, tag="xn", name="xn")
                nc.vector.tensor_scalar(xn, x, mv[:, 0:1], y,
                                        ALU.subtract, ALU.mult)
                nc.vector.tensor_mul(xn, xn, scale_sb)
                nc.vector.tensor_add(xn, xn, offset_sb)
                nc.sync.dma_start(
                    out_ap[qc * P:(qc + 1) * P, :],
                    xn.rearrange("p a b -> p (a b)"),
                )

            # last head-pair chunk: final-proj matmuls+stats AND their LN
            # chains for sq0 pumped into the second half's sk stream (their
            # DVE work overlaps the ACT-paced attention); sq1's groups run
            # after (their ot_sb columns come from this call's epilogue)
            c = NF - 1
            attn(c, 0, [])

            def final_full(qc):
                def emit():
                    final_mm(qc)()
                    final_ln(qc)
                return emit

            # sq0 groups' full chains pumped into attn(7,1): their DVE work
            # (now bf16 + early-psum-free epilogue) fits in the attention
            # window without backing up the epilogue queue
            work = [final_full(qc) for qc in range(4)]
            attn(c, 1, work, pump_sks=PUMP_SKS_LAST)
            for w in work:
                w()
            for qc in range(4, SQ // P):
                final_mm(qc)()
            for qc in range(4, SQ // P):
                final_ln(qc)


def build_program():
    nc = bacc.Bacc("TRN2", debug=False, target_bir_lowering=False)
    shapes = {
        "qT": ([D, SQ], BF16), "kT": ([D, SK], BF16), "vT": ([D, SK], BF16),
        "qres": ([SQ, D], BF16),
        "wqT": ([D, F], BF16), "wkT": ([D, F], BF16), "wvT": ([D, F], BF16),
        "wpT": ([F, D], BF16),
        "scale_b": ([P, D], BF16), "offset_b": ([P, D], BF16),
    }
    ins = {k: nc.dram_tensor(k, shp, dt, kind="ExternalInput").ap()
           for k, (shp, dt) in shapes.items()}
    out = nc.dram_tensor("out", [SQ, D], BF16, kind="ExternalOutput").ap()
    with tile.TileContext(nc) as tc:
        _mha_kernel(tc, out, ins)
    nc.compile()
    return nc


_PROGRAM = None


def _get_program():
    global _PROGRAM
    if _PROGRAM is None:
        _PROGRAM = build_program()
    return _PROGRAM


def make_in_maps(q, k, v, Wq, Wk, Wv, Wp, scale, offset):
    import ml_dtypes
    f = np.float32
    bf = ml_dtypes.bfloat16
    q = np.asarray(q, f)
    k16 = np.asarray(k, f).astype(bf)
    v16 = np.asarray(v, f).astype(bf)
    q16 = q.astype(bf)
    wqT = np.ascontiguousarray(
        np.asarray(Wq, f).transpose(2, 0, 1).reshape(D, F).astype(bf))
    wkT = np.ascontiguousarray(
        np.asarray(Wk, f).transpose(2, 0, 1).reshape(D, F).astype(bf))
    wvT = np.ascontiguousarray(
        np.asarray(Wv, f).transpose(2, 0, 1).reshape(D, F).astype(bf))
    wpT = np.ascontiguousarray(np.asarray(Wp, f).T.astype(bf))
    scale_b = np.ascontiguousarray(
        np.broadcast_to(np.asarray(scale, f), (P, D)).astype(bf))
    offset_b = np.ascontiguousarray(
        np.broadcast_to(np.asarray(offset, f), (P, D)).astype(bf))
    in_maps = []
    for c in range(N_CORES):
        b, half = divmod(c, 2)
        sl = slice(half * SQ, (half + 1) * SQ)
        in_maps.append({
            "qT": np.ascontiguousarray(q16[b, sl].T),
            "qres": np.ascontiguousarray(q16[b, sl]),
            "kT": np.ascontiguousarray(k16[b].T),
            "vT": np.ascontiguousarray(v16[b].T),
            "wqT": wqT, "wkT": wkT, "wvT": wvT, "wpT": wpT,
            "scale_b": scale_b, "offset_b": offset_b,
        })
    return in_maps


def kernel(q, k, v, Wq, Wk, Wv, Wp, scale, offset):
    global LAST_RESULT
    in_maps = make_in_maps(q, k, v, Wq, Wk, Wv, Wp, scale, offset)
    nc = _get_program()
    res = run_bass_kernel_spmd(nc, in_maps, list(range(N_CORES)))
    LAST_RESULT = res
    out = np.empty((B, S, D), np.float32)
    for c in range(N_CORES):
        b, half = divmod(c, 2)
        out[b, half * SQ:(half + 1) * SQ] = \
            res.results[c]["out"].astype(np.float32)
    return out

